# revision 1
# baseline (speedup 1.0000x reference)
"""Trainium2 Bass kernel for nn_MHA_63118839382398.

Full MHA block: fused QKV projection, per-head RMSNorm on q/k, rotate-half
RoPE, causal softmax attention, output projection.

Sharding over 8 NeuronCores: core c handles batch b = c//2 and heads
[8*(c%2), 8*(c%2)+8) (tensor parallel over head halves within a batch
pair). Each core computes a partial out-projection; a 2-rank
ReduceScatter (bf16) over each pair sums the partials and leaves each
core with half of that batch's token rows, which the host reassembles.

Layout strategy (all transposed, feats x tokens), so every matmul
contraction sits on the partition axis with no on-chip transposes except
V (cheap PE-mode 128x128 transposes).

Structure (two phases, engine-balance driven; ~430us vs 554us baseline):
  P phase: projections + rms + rope for ALL 4 head-pair waves,
    software-pipelined so every PE round trip (sumsq -> Ln/Exp -> fac;
    vt -> transposes) hides under the next chunk's projection streams
    and HAM stays warm. Psum evacuations ride ACT (Square/Copy) and DVE
    tensor_scalar (rms gains as the per-partition scalar).
  A phase: attention in query-chunk-major order (qc outer, wave inner):
    the two head halves' score matmuls are row-group concurrent
    (tile_position via 64-row base partitions) and land in one 2-bank
    PSUM tile so ONE exp instruction covers both (ACT cost is free-size
    x 0.833ns + ~430ns fixed, so batching the free dim is everything).
    PV runs 2 key-blocks behind the scores; the softmax epilogue's
    1/denominator is exp(-ln(den)) on ACT with a gpsimd
    partition_broadcast; per-qc yt tiles avoid false cross-qc deps.
    Each qc's out-projection + 2-rank ReduceScatter spreads into the
    next qc's attention as PE filler (collectives fully overlap);
    the post-RS f32 unpack is a single casting DRAM->DRAM gpsimd DMA
    deferred 2 chunks so no engine queue ever blocks on a collective.
    The last chunk uses two half-RS so the first overlaps the second's
    out-proj units; a tiny warm-up collective absorbs the cc-stream
    spin-up during the P phase.
"""

import sys

if "/opt/trn_rl_repo" not in sys.path:
    sys.path.insert(0, "/opt/trn_rl_repo")

import numpy as np
import ml_dtypes

import concourse.bass as bass
import concourse.tile as tile
from concourse import bacc, mybir
from concourse.bass_utils import run_bass_kernel_spmd
from concourse.masks import make_identity

# Problem constants (hardcoded per harness contract).
B = 4
N = 2048
D_MODEL = 1024
N_HEADS = 16
D_HEAD = 64
ROPE_BASE = 10000.0
EPS = float(np.finfo(np.float32).eps)
N_CORES = 8

HPC = N_HEADS // 2          # heads per core = 8
WAVES = HPC // 2            # head-pair waves = 4
TOKCH = 512                 # token chunk for projections / q chunks
NT = N // TOKCH             # 4
QT = 128                    # query tile for mask classification
NQT = N // QT               # 16
KB = 128                    # key block
NKB = N // KB               # 16
DC = 128                    # dmodel chunk
NDC = D_MODEL // DC         # 8

F32 = mybir.dt.float32
BF16 = mybir.dt.bfloat16
BF = ml_dtypes.bfloat16

ACT = mybir.ActivationFunctionType

_CACHE = {}


def _pin_act_tables(arch):
    """Steer bacc's ACT-table-set choice to natural_log_exp_and_others.

    The insertion pass picks the first set containing each activation's
    function; removing our functions from every other set's selection
    metadata makes Copy/Square/Ln/Exp resolve to the one set that has
    them all (no mid-kernel table reloads)."""
    from concourse.hw_specs import get_activation_tables

    tables = get_activation_tables(arch)  # cached by reference
    keep = "natural_log_exp_and_others"
    if keep not in tables:
        return
    ours = {ACT.Copy, ACT.Square, ACT.Ln, ACT.Exp, ACT.Identity}
    for name, fns in tables.items():
        if name != keep:
            fns -= ours


def _classify_mask(mask):
    """Per (key-block, query-tile) classification of the mask.

    Returns (state[NKB][NQT], patterns): state is 'skip' (all masked),
    'full' (none masked), or an index into patterns: unique (128,128)
    bf16 0/1 tiles indexed [key, query]."""
    mask = np.asarray(mask)
    assert mask.shape == (N, N)
    patterns = []
    pat_keys = {}
    state = [[None] * NQT for _ in range(NKB)]
    for kb in range(NKB):
        for qt in range(NQT):
            blk = mask[qt * QT : (qt + 1) * QT, kb * KB : (kb + 1) * KB]
            if blk.all():
                state[kb][qt] = "skip"
            elif not blk.any():
                state[kb][qt] = "full"
            else:
                tileq = (~blk.T).astype(BF)
                key = tileq.tobytes()
                if key not in pat_keys:
                    pat_keys[key] = len(patterns)
                    patterns.append(tileq)
                state[kb][qt] = pat_keys[key]
    return state, patterns


def _build_program(state, n_patterns):
    """Build the SPMD Bass program (same graph on all 8 cores)."""
    nc = bacc.Bacc(
        "TRN2", target_bir_lowering=False, debug=False, num_devices=N_CORES
    )
    _pin_act_tables(nc.m.arch)

    p_xt = nc.dram_tensor("xt", [D_MODEL, N], BF16, kind="ExternalInput").ap()
    p_wqk = nc.dram_tensor("wqk", [128, 2, WAVES, NDC, 128], BF16, kind="ExternalInput").ap()
    p_wv = nc.dram_tensor("wv", [128, WAVES, NDC, 128], BF16, kind="ExternalInput").ap()
    p_wo = nc.dram_tensor("wo", [128, 4, D_MODEL], BF16, kind="ExternalInput").ap()
    # rope tables (gain-free, shared by q/k): [128, cos/sin, N]
    p_rope = nc.dram_tensor("rope", [128, 2, N], BF16, kind="ExternalInput").ap()
    p_wcol = nc.dram_tensor("wcol", [128, 2], F32, kind="ExternalInput").ap()
    p_ind2 = nc.dram_tensor("ind2", [128, 2], BF16, kind="ExternalInput").ap()
    p_wfold = nc.dram_tensor("wfold", [2, 128], BF16, kind="ExternalInput").ap()
    p_pswap = nc.dram_tensor("pswap", [128, 128], BF16, kind="ExternalInput").ap()
    if n_patterns:
        p_pat = nc.dram_tensor(
            "pat", [128, n_patterns, 128], BF16, kind="ExternalInput"
        ).ap()
    p_out = nc.dram_tensor("out", [N // 2, D_MODEL], F32, kind="ExternalOutput").ap()

    y_parts = [
        nc.dram_tensor(f"y_part{qc}", [TOKCH, D_MODEL], BF16) for qc in range(NT)
    ]
    rs_outs = [
        nc.dram_tensor(f"rs_out{qc}", [256, D_MODEL], BF16) for qc in range(NT)
    ]
    # the last chunk's out-proj is the kernel tail: use 2 half-RS so the
    # first half overlaps the second half's units
    y3h = [nc.dram_tensor(f"y3h{h}", [256, D_MODEL], BF16) for h in range(2)]
    rs3h = [nc.dram_tensor(f"rs3h{h}", [128, D_MODEL], BF16) for h in range(2)]
    # tiny dummy collective to absorb the ~11.5us cc-stream spin-up
    cc_warm_in = nc.dram_tensor("ccw_in", [2, 128], BF16)
    cc_warm_out = nc.dram_tensor("ccw_out", [1, 128], BF16)

    QPC = TOKCH // QT  # query tiles per chunk = 4
    n_kb = [0] * NT
    qlo_t = {}
    for qc in range(NT):
        for kb in range(NKB):
            sub = [state[kb][qc * QPC + j] for j in range(QPC)]
            if all(s == "skip" for s in sub):
                continue
            n_kb[qc] = max(n_kb[qc], kb + 1)
            lead = 0
            while sub[lead] == "skip":
                lead += 1
            qlo_t[(qc, kb)] = lead

    with tile.TileContext(nc) as tc:
        import contextlib

        ctx = contextlib.ExitStack()
        with ctx:
            singles = ctx.enter_context(tc.tile_pool(name="singles", bufs=1))
            wavep = ctx.enter_context(tc.tile_pool(name="wavep", bufs=2))
            invp = ctx.enter_context(tc.tile_pool(name="invp", bufs=2))
            work = ctx.enter_context(tc.tile_pool(name="work", bufs=2))
            espool = ctx.enter_context(tc.tile_pool(name="es", bufs=4))
            epi = ctx.enter_context(tc.tile_pool(name="epi", bufs=1))
            outp = ctx.enter_context(tc.tile_pool(name="outp", bufs=2))

            # PSUM budget (8 banks): ps tag "s" 3x[128,2,512]f32 = 6 banks,
            # po 1x[128,2,512] = 2 banks.
            ps = ctx.enter_context(tc.tile_pool(name="ps", bufs=3, space="PSUM"))
            ppo = ctx.enter_context(tc.tile_pool(name="ppo", bufs=1, space="PSUM"))

            # ---- resident constants -------------------------------------
            xt_sb = [singles.tile([128, NDC, TOKCH], BF16, name=f"xt{t}") for t in range(NT)]
            wqk_sb = [
                singles.tile([128, 2, NDC, 128], BF16, name=f"wqk{w}")
                for w in range(WAVES)
            ]
            wv_sb = [
                singles.tile([128, NDC, 128], BF16, name=f"wv{w}")
                for w in range(WAVES)
            ]
            rope_sb = singles.tile([128, 2, N], BF16)
            wcol = singles.tile([128, 2], F32)
            ident = singles.tile([128, 128], BF16)
            make_identity(nc, ident)
            eps_sb = singles.tile([128, 1], F32)
            nc.vector.memset(eps_sb, EPS)
            pswap = singles.tile([128, 128], BF16)
            ind2 = singles.tile([128, 2], BF16)
            wfold = singles.tile([2, 128], BF16)
            if n_patterns:
                pat_sb = singles.tile([128, n_patterns, 128], BF16)
            yt_sb = [
                singles.tile([128, WAVES, TOKCH], BF16, name=f"yt{qc}")
                for qc in range(NT)
            ]
            wo_sb = singles.tile([128, 4, D_MODEL], BF16)
            # per-wave persistent attention operands
            qk_rot = [
                singles.tile([128, 2, N], BF16, name=f"qkrot{w}")
                for w in range(WAVES)
            ]
            v_sb = [
                singles.tile([128, NKB, 130], BF16, name=f"vsb{w}")
                for w in range(WAVES)
            ]

            # ---- initial DMAs: round-robin across the 3 queues in need
            # order so the front of the P phase is fed as fast as the
            # aggregate DMA bandwidth allows
            dma_jobs = []  # (dst_ap, src_ap, approx_bytes)
            dma_jobs.append((wcol, p_wcol, 1))
            dma_jobs.append((ind2, p_ind2, 1))
            dma_jobs.append((wfold, p_wfold, 1))
            dma_jobs.append((pswap, p_pswap, 32))
            for qk in range(2):
                dma_jobs.append((wqk_sb[0][:, qk], p_wqk[:, qk, 0], 256))
            dma_jobs.append((wv_sb[0], p_wv[:, 0, :, :], 256))
            for dc in range(NDC):
                dma_jobs.append(
                    (xt_sb[0][:, dc, :], p_xt[dc * DC : (dc + 1) * DC, 0:TOKCH], 128)
                )
            # rope halves (needed by tail(0,0) early)
            dma_jobs.append((rope_sb[:, :, 0:N // 2], p_rope[:, :, 0:N // 2], 512))
            dma_jobs.append((rope_sb[:, :, N // 2:], p_rope[:, :, N // 2:], 512))
            for w2 in range(1, WAVES):
                t2 = w2
                for qk in range(2):
                    dma_jobs.append((wqk_sb[w2][:, qk], p_wqk[:, qk, w2], 256))
                dma_jobs.append((wv_sb[w2], p_wv[:, w2, :, :], 256))
                for dc in range(NDC):
                    dma_jobs.append(
                        (
                            xt_sb[t2][:, dc, :],
                            p_xt[dc * DC : (dc + 1) * DC, t2 * TOKCH : (t2 + 1) * TOKCH],
                            128,
                        )
                    )
            if n_patterns:
                dma_jobs.append((pat_sb, p_pat, 1))
            dma_jobs.append((wo_sb, p_wo, 1024))
            # scalar-engine DMA triggers delay the P phase's ACT compute
            # ops behind them, so the scalar queue gets only a small,
            # late-needed share; the rest round-robins sync/gpsimd.
            qs = [nc.sync, nc.gpsimd]
            qload = [0, 0]
            scalar_budget = 5
            for dst, src, kb in dma_jobs:
                if kb >= 256 and scalar_budget > 0 and qload[0] > 1024:
                    nc.scalar.dma_start(out=dst, in_=src)
                    scalar_budget -= 1
                    continue
                qi = qload.index(min(qload))
                qs[qi].dma_start(out=dst, in_=src)
                qload[qi] += kb
            # collective stream warm-up: emitted AFTER the gpsimd-queue
            # DMAs -- its trigger waits the ~20us cc-stream init barrier
            # and would block every DMA queued behind it
            nc.gpsimd.collective_compute(
                "ReduceScatter",
                mybir.AluOpType.add,
                ins=[cc_warm_in.ap().opt()],
                outs=[cc_warm_out.ap().opt()],
                replica_groups=[[0, 1], [2, 3], [4, 5], [6, 7]],
            )
            for w in range(WAVES):
                nc.vector.memset(v_sb[w][:, :, 64:65], 1.0)
                nc.vector.memset(v_sb[w][:, :, 129:130], 1.0)
            # =============== P phase: proj + rms + rope ==================
            # Split into proj part (pure PE streaming) and tail part (the
            # rms/rope dependency chain); tail(c) is emitted after
            # proj(c+1) so its ACT/DVE latency hides under the next
            # chunk's matmul stream.
            def emit_P_proj(w, t):
                pj = ps.tile([128, 2, TOKCH], F32, tag="s", name="pj")
                for qk in range(2):
                    for dc in range(NDC):
                        nc.tensor.matmul(
                            pj[:, qk, :],
                            lhsT=wqk_sb[w][:, qk, dc, :],
                            rhs=xt_sb[t][:, dc, :],
                            start=(dc == 0),
                            stop=(dc == NDC - 1),
                        )
                pjv = ps.tile([128, 2, TOKCH], F32, tag="s", name="pjv")
                for dc in range(NDC):
                    nc.tensor.matmul(
                        pjv[:, 0, :],
                        lhsT=wv_sb[w][:, dc, :],
                        rhs=xt_sb[t][:, dc, :],
                        start=(dc == 0),
                        stop=(dc == NDC - 1),
                    )
                return pj, pjv

            def emit_P_evac(w, t, pj, pjv):
                """Emit right after chunk (w,t)'s projection MMs: psum
                evacuation on DVE/ACT. Gains ride the per-partition scalar
                of tensor_scalar."""
                raw = wavep.tile([128, 2, TOKCH], BF16, tag="raw", name="raw")
                for qk in range(2):
                    nc.vector.tensor_scalar_mul(
                        raw[:, qk, :], pj[:, qk, :], wcol[:, qk : qk + 1]
                    )
                sq = work.tile([128, 2, TOKCH], BF16, tag="sq")
                nc.scalar.square(sq, pj)          # ACT (pre-gain squares)
                vt = work.tile([128, TOKCH], BF16, tag="vt")
                nc.scalar.copy(vt, pjv[:, 0, :])  # ACT
                return raw, sq, vt

            def emit_P_rms(w, t, sq):
                """Sumsq reduce + batched Ln/Exp. Emitted at the START of
                the next iteration: the 2 ssp MMs run before the next
                projection stream, so the Ln/Exp complete ~3us before the
                fac matmuls need them."""
                lnm = work.tile([2, 2, TOKCH], BF16, tag="lnm")
                inv = invp.tile([2, 2, TOKCH], BF16, tag="inv", name="inv")
                ssp = ps.tile([2, 2, TOKCH], F32, tag="s", name="ssp")
                for qk in range(2):
                    nc.tensor.matmul(
                        ssp[:, qk, :], lhsT=ind2, rhs=sq[:, qk, :],
                        start=True, stop=True,
                    )
                nc.scalar.activation(
                    lnm, ssp, ACT.Ln, bias=eps_sb[0:2, :], scale=1.0 / D_HEAD
                )
                nc.scalar.activation(inv, lnm, ACT.Exp, scale=-0.5)
                return inv

            def emit_P_vtrans(w, t, vt):
                ptr = ps.tile([128, 4, 128], BF16, tag="s", name="ptr")
                for sview in range(4):
                    nc.tensor.transpose(
                        ptr[:, sview, :],
                        vt[:, sview * 128 : (sview + 1) * 128],
                        ident,
                    )
                kb0 = t * 4
                nc.vector.tensor_copy(
                    v_sb[w][:, kb0 : kb0 + 4, 0:64], ptr[:, :, 0:64]
                )
                nc.vector.tensor_copy(
                    v_sb[w][:, kb0 : kb0 + 4, 65:129], ptr[:, :, 64:128]
                )

            def emit_P_rope(w, t, raw, inv):
                """fac/swap matmuls + rope muls (emitted after proj_v of
                the NEXT chunk so the Ln/Exp latency hides)."""
                tsl = slice(t * TOKCH, (t + 1) * TOKCH)
                qn = work.tile([128, 2, TOKCH], BF16, tag="qn")
                qcos = work.tile([128, 2, TOKCH], BF16, tag="qcos")
                qsin = work.tile([128, 2, TOKCH], BF16, tag="qsin")
                fsw = [None, None]
                for qk in range(2):
                    fsw[qk] = ps.tile([128, 2, TOKCH], F32, tag="s", name="fsw")
                    nc.tensor.matmul(
                        fsw[qk][:, 0, :], lhsT=wfold, rhs=inv[:, qk, :],
                        start=True, stop=True,
                    )
                    nc.vector.tensor_mul(
                        qn[:, qk, :], raw[:, qk, :], fsw[qk][:, 0, :]
                    )
                for qk in range(2):
                    nc.tensor.matmul(
                        fsw[qk][:, 1, :], lhsT=pswap, rhs=qn[:, qk, :],
                        start=True, stop=True,
                    )
                    nc.vector.tensor_mul(
                        qcos[:, qk, :], qn[:, qk, :], rope_sb[:, 0, tsl]
                    )
                    nc.vector.tensor_mul(
                        qsin[:, qk, :], fsw[qk][:, 1, :], rope_sb[:, 1, tsl]
                    )
                nc.vector.tensor_add(qk_rot[w][:, :, tsl], qcos, qsin)

            # =============== A phase: attention, qc-major ================
            def emit_D(qc, w, prologue):
                """Attention for (qc, w). `prologue` is a list of closures
                (previous wave's epilogue, out-proj filler units) emitted
                between the first score pairs and the first PV so the PE
                FIFO never stalls on their dependencies. Returns this
                wave's epilogue closure."""
                kbs = [kb for kb in range(n_kb[qc]) if (qc, kb) in qlo_t]
                po = ppo.tile([128, 2, TOKCH], F32, tag="po", name="po")
                first = [True, True]
                pend = []

                def flush_pv(kb, es, last):
                    qlo = qlo_t[(qc, kb)] * QT
                    osl = slice(qlo, TOKCH)
                    for h2 in range(2):
                        nc.tensor.matmul(
                            po[0:65, h2, osl],
                            lhsT=v_sb[w][:, kb, 65 * h2 : 65 * h2 + 65],
                            rhs=es[:, h2, osl],
                            start=first[h2],
                            stop=last,
                        )
                        first[h2] = False

                for i, kb in enumerate(kbs):
                    qlo = qlo_t[(qc, kb)] * QT
                    csl = slice(qc * TOKCH + qlo, (qc + 1) * TOKCH)
                    osl = slice(qlo, TOKCH)
                    pst = ps.tile([128, 2, TOKCH], F32, tag="s", name="pst")
                    for h2 in range(2):
                        hr = slice(64 * h2, 64 * h2 + 64)
                        nc.tensor.matmul(
                            pst[:, h2, osl],
                            lhsT=qk_rot[w][hr, 1, kb * KB : (kb + 1) * KB],
                            rhs=qk_rot[w][hr, 0, csl],
                            start=True,
                            stop=True,
                        )
                    es = espool.tile([128, 2, TOKCH], BF16, tag="es", name="es")
                    nc.scalar.activation(
                        es[:, :, osl], pst[:, :, osl], ACT.Exp,
                        scale=float(D_HEAD) ** -0.5,
                    )
                    for j in range(qlo // QT, QPC):
                        st = state[kb][qc * QPC + j]
                        if isinstance(st, int):
                            jsl = slice(j * QT, (j + 1) * QT)
                            for h2 in range(2):
                                nc.vector.tensor_mul(
                                    es[:, h2, jsl], es[:, h2, jsl],
                                    pat_sb[:, st, :],
                                )
                    if prologue:
                        prologue.pop(0)()
                    pend.append((kb, es))
                    if len(pend) > 2:
                        k0, e0 = pend.pop(0)
                        flush_pv(k0, e0, False)
                for fn in prologue:
                    fn()
                for i, (k0, e0) in enumerate(pend):
                    flush_pv(k0, e0, i == len(pend) - 1)

                def epilogue():
                    # po rows 0:63 = y_raw, row 64 = softmax denominator;
                    # 1/den = exp(-ln(den)) on ACT (single-partition DVE
                    # reciprocal is 6.5us and clogs the DVE FIFO)
                    yr = epi.tile([64, 2, TOKCH], BF16, tag="yr", name="yr")
                    nc.vector.tensor_copy(yr, po[0:64, :, :])
                    lnd = epi.tile([1, 2, TOKCH], F32, tag="lnd", name="lnd")
                    nc.scalar.activation(lnd, po[64:65, :, :], ACT.Ln)
                    recb = epi.tile([1, 2, TOKCH], BF16, tag="recb", name="recb")
                    nc.scalar.activation(recb, lnd, ACT.Exp, scale=-1.0)
                    f2 = epi.tile([64, 2, TOKCH], BF16, tag="f2", name="f2")
                    for h2 in range(2):
                        nc.gpsimd.partition_broadcast(
                            f2[:, h2, :], recb[:, h2, :]
                        )
                    for h2 in range(2):
                        nc.vector.tensor_mul(
                            yt_sb[qc][
                                64 * h2 : 64 * h2 + 64, w, :
                            ],
                            yr[:, h2, :],
                            f2[:, h2, :],
                        )

                return epilogue

            def out_unit(qc, i):
                """One quarter of qc's out-projection: 128 tokens x 1024
                out-features -> y_parts[qc][i//2]."""
                def fn():
                    pot = ps.tile([128, 2, TOKCH], F32, tag="s", name="pot")
                    for ec in range(2):
                        for fc in range(4):
                            nc.tensor.matmul(
                                pot[:, ec, :],
                                lhsT=yt_sb[qc][:, fc, i * 128 : (i + 1) * 128],
                                rhs=wo_sb[:, fc, ec * TOKCH : (ec + 1) * TOKCH],
                                start=(fc == 0),
                                stop=(fc == 3),
                            )
                    osb = outp.tile([128, 2, TOKCH], BF16, tag="o", name="osb")
                    nc.vector.tensor_copy(osb, pot)
                    nc.sync.dma_start(
                        out=y_parts[qc].ap()[i * 128 : (i + 1) * 128, :], in_=osb
                    )
                return fn

            def rs_trigger(qc):
                def fn():
                    nc.gpsimd.collective_compute(
                        "ReduceScatter",
                        mybir.AluOpType.add,
                        ins=[y_parts[qc].ap().opt()],
                        outs=[rs_outs[qc].ap().opt()],
                        replica_groups=[[0, 1], [2, 3], [4, 5], [6, 7]],
                    )
                return fn

            def unpack(qc):
                """Post-RS f32 unpack: one casting DRAM->DRAM DMA on the
                software DGE (gpsimd is the only engine that can cast in a
                DMA). Scheduled >=1 full qc after the RS so the gpsimd
                queue never blocks on the collective."""
                def fn():
                    nc.gpsimd.dma_start(
                        out=p_out[qc * 256 : (qc + 1) * 256, :],
                        in_=rs_outs[qc].ap(),
                    )
                return fn

            # ---------------- emission schedule --------------------------
            # P phase, software-pipelined: chunk c's dependent chain is
            # split so each PE round trip (ssp -> Ln/Exp -> fac -> qn ->
            # swp) hides under the next chunk's projection streams.
            chunks = [(w, t) for w in range(WAVES) for t in range(NT)]
            pend = None  # (w, t, raw, sq, vt)
            for w, t in chunks:
                if pend is not None:
                    pw, pt, raw_p, sq_p, vt_p = pend
                    inv_p = emit_P_rms(pw, pt, sq_p)
                pj, pjv = emit_P_proj(w, t)
                raw, sq, vt = emit_P_evac(w, t, pj, pjv)
                if pend is not None:
                    emit_P_vtrans(pw, pt, vt_p)
                    emit_P_rope(pw, pt, raw_p, inv_p)
                pend = (w, t, raw, sq, vt)
            pw, pt, raw_p, sq_p, vt_p = pend
            inv_p = emit_P_rms(pw, pt, sq_p)
            emit_P_vtrans(pw, pt, vt_p)
            emit_P_rope(pw, pt, raw_p, inv_p)

            # A phase: out-proj of qc spreads into the next qc as filler
            filler = []
            rs_fired = []   # qcs whose RS has been triggered, oldest first
            prev_epi = None
            for qc in [0, 1, 2, 3]:
                for w in range(WAVES):
                    prologue = []
                    if prev_epi is not None:
                        prologue.append(prev_epi)
                        prev_epi = None
                    for _ in range(2):
                        if filler:
                            prologue.append(filler.pop(0))
                    prev_epi = emit_D(qc, w, prologue)
                assert not filler, f"filler left over at qc={qc}"
                if qc != NT - 1:
                    filler = [out_unit(qc, i) for i in range(4)] + [rs_trigger(qc)]
                    rs_fired.append(qc)
                    if len(rs_fired) > 2:
                        filler.append(unpack(rs_fired.pop(0)))
                else:
                    # tail: half-RS pipeline, casts on the now-idle ACT
                    prev_epi()
                    for i in range(4):
                        pot = ps.tile([128, 2, TOKCH], F32, tag="s", name="pot")
                        for ec in range(2):
                            for fc in range(4):
                                nc.tensor.matmul(
                                    pot[:, ec, :],
                                    lhsT=yt_sb[qc][:, fc, i * 128 : (i + 1) * 128],
                                    rhs=wo_sb[:, fc, ec * TOKCH : (ec + 1) * TOKCH],
                                    start=(fc == 0),
                                    stop=(fc == 3),
                                )
                        osb = outp.tile([128, 2, TOKCH], BF16, tag="o", name="osb")
                        nc.scalar.copy(osb, pot)
                        nc.sync.dma_start(
                            out=y3h[i // 2].ap()[(i % 2) * 128 : (i % 2) * 128 + 128, :],
                            in_=osb,
                        )
                        if i == 1 or i == 3:
                            h = i // 2
                            nc.gpsimd.collective_compute(
                                "ReduceScatter",
                                mybir.AluOpType.add,
                                ins=[y3h[h].ap().opt()],
                                outs=[rs3h[h].ap().opt()],
                                replica_groups=[[0, 1], [2, 3], [4, 5], [6, 7]],
                            )
                        if i == 3:
                            for q2 in rs_fired:
                                unpack(q2)()
                            for h in range(2):
                                r0 = (NT - 1) * 256 + h * 128
                                nc.gpsimd.dma_start(
                                    out=p_out[r0 : r0 + 128, :],
                                    in_=rs3h[h].ap(),
                                )

    nc.compile()
    return nc


def _host_prep(x, mask, pos, W_qkv, W_out, qn_w, kn_w):
    x = np.asarray(x, dtype=np.float32)
    mask = np.asarray(mask)
    pos = np.asarray(pos).astype(np.float64)
    W_qkv = np.asarray(W_qkv, dtype=np.float32)
    W_out = np.asarray(W_out, dtype=np.float32)
    qn_w = np.asarray(qn_w, dtype=np.float32)
    kn_w = np.asarray(kn_w, dtype=np.float32)

    inv_freq = 1.0 / (ROPE_BASE ** (np.arange(0, D_HEAD, 2, dtype=np.float64) / D_HEAD))
    ang = pos[:, None] * inv_freq[None, :]  # (N, 32)
    cosT = np.cos(ang).T.astype(np.float32)  # (32, N)
    sinT = np.sin(ang).T.astype(np.float32)

    # gain-free tables shared by q and k (gains applied via per-partition
    # ACT scale on the raw copies)
    cos_d = np.tile(cosT, (4, 1))
    sin_d = np.tile(np.concatenate([-sinT, sinT], axis=0), (2, 1))
    rope = np.stack([cos_d, sin_d], axis=1).astype(BF)  # (128, 2, N)
    wcol_np = np.stack([np.tile(qn_w, 2), np.tile(kn_w, 2)], axis=1).astype(
        np.float32
    )  # (128, 2)

    pswap_np = np.zeros((128, 128), dtype=np.float32)
    for a in range(2):
        for r in range(32):
            pswap_np[64 * a + r, 64 * a + 32 + r] = 1.0
            pswap_np[64 * a + 32 + r, 64 * a + r] = 1.0
    pswap_np = pswap_np.astype(BF)

    ind2_np = np.zeros((128, 2), dtype=np.float32)
    ind2_np[0:64, 0] = 1.0
    ind2_np[64:128, 1] = 1.0
    ind2_np = ind2_np.astype(BF)
    wfold_np = np.ascontiguousarray(ind2_np.T)  # (2, 128)

    state, patterns = _classify_mask(mask)
    if patterns:
        pat = np.stack(patterns, axis=1).astype(BF)
    else:
        pat = None

    q_rows = lambda h: slice(h * 192, h * 192 + 64)
    k_rows = lambda h: slice(h * 192 + 64, h * 192 + 128)
    v_rows = lambda h: slice(h * 192 + 128, h * 192 + 192)

    in_maps = []
    for c in range(N_CORES):
        b, half = divmod(c, 2)
        hs = [8 * half + i for i in range(8)]
        wqk = np.concatenate(
            [W_qkv[q_rows(h)] for h in hs] + [W_qkv[k_rows(h)] for h in hs], axis=0
        ).T  # (1024 dmodel, 1024 cols)
        wv = np.concatenate([W_qkv[v_rows(h)] for h in hs], axis=0).T
        wo = W_out[:, 512 * half : 512 * half + 512].T  # (512, 1024)
        # (128, 2, WAVES, NDC, 128): [p, qk, w, dc, f]
        wqk_re = np.ascontiguousarray(
            wqk.reshape(NDC, 128, 2, WAVES, 128).transpose(1, 2, 3, 0, 4)
        )
        wv_re = np.ascontiguousarray(
            wv.reshape(NDC, 128, WAVES, 128).transpose(1, 2, 0, 3)
        )
        wo_re = np.ascontiguousarray(wo.reshape(4, 128, 1024).transpose(1, 0, 2))
        m = {
            "xt": np.ascontiguousarray(x[b].T).astype(BF),
            "wqk": wqk_re.astype(BF),
            "wv": wv_re.astype(BF),
            "wo": wo_re.astype(BF),
            "rope": rope,
            "wcol": wcol_np,
            "ind2": ind2_np,
            "wfold": wfold_np,
            "pswap": pswap_np,
        }
        if pat is not None:
            m["pat"] = pat
        in_maps.append(m)
    return in_maps, state, (0 if pat is None else pat.shape[1])


def kernel(x, mask, pos, W_qkv, W_out, qn_w, kn_w, _trace=False):
    in_maps, state, n_pat = _host_prep(x, mask, pos, W_qkv, W_out, qn_w, kn_w)
    key = (str(state), n_pat)
    if key not in _CACHE:
        _CACHE[key] = _build_program(state, n_pat)
    nc = _CACHE[key]
    res = run_bass_kernel_spmd(nc, in_maps, list(range(N_CORES)), trace=_trace)
    out = np.empty((B, N, D_MODEL), dtype=np.float32)
    for b in range(B):
        lo = res.results[2 * b]["out"]
        hi = res.results[2 * b + 1]["out"]
        for qc in range(NT - 1):
            out[b, qc * TOKCH : qc * TOKCH + 256] = lo[qc * 256 : (qc + 1) * 256]
            out[b, qc * TOKCH + 256 : (qc + 1) * TOKCH] = hi[qc * 256 : (qc + 1) * 256]
        # last chunk used 2 half-ReduceScatters: each 256-row half-RS
        # leaves rank0 rows 0:128, rank1 rows 128:256
        qc = NT - 1
        for h in range(2):
            t0 = qc * TOKCH + h * 256
            r0 = qc * 256 + h * 128
            out[b, t0 : t0 + 128] = lo[r0 : r0 + 128]
            out[b, t0 + 128 : t0 + 256] = hi[r0 : r0 + 128]
    kernel._last_results = res
    return out



# revision 4
# speedup vs baseline: 1.1906x; 1.1906x over previous
"""Trainium2 Bass kernel for nn_MHA_63118839382398.

Full MHA block: fused QKV projection, per-head RMSNorm on q/k, rotate-half
RoPE, causal softmax attention, output projection.

Sharding over 8 NeuronCores: core c handles batch b = c//2 and heads
[8*(c%2), 8*(c%2)+8) (tensor parallel over head halves within a batch
pair). Each core computes a partial out-projection over its 8 heads and
writes the bf16 partial [2048, 1024] to DRAM; the HOST sums the two
partials of each batch pair (identical numerics to the on-device CCE
add of bf16 partials, but avoids the ~20us-floor mesh collectives that
serialized the pipeline and made a 40us kernel tail).

Layout strategy (all transposed, feats x tokens), so every matmul
contraction sits on the partition axis with no on-chip transposes except
V (cheap PE-mode 128x128 transposes).

Structure (two phases):
  P phase: projections + rms + rope for ALL 4 head-pair waves,
    software-pipelined so every PE round trip (sumsq -> Ln/Exp -> fac)
    hides under the next chunk's projection streams. Psum evacuations
    ride ACT (Square/Copy) and DVE tensor_scalar. The rotate-half swap
    runs on DVE stream_shuffle (head-dim rows are host-permuted so the
    rope partner lives 16 rows away inside the same 32-partition
    quadrant), keeping the PE stream pure projection work.
  A phase: attention in query-chunk-major order (qc outer, wave inner):
    the two head halves' score matmuls are row-group concurrent
    (tile_position via 64-row base partitions) and land in one 2-bank
    PSUM tile so ONE exp instruction covers both. PV runs 2 key-blocks
    behind the scores. Softmax epilogue: each wave's denominator row
    (the 65th ones-row of V) is copied to one partition of a 4-wave
    staging tile; ONE Ln + ONE Exp per qc computes all 4 waves'
    reciprocals ([4,2,512] on ACT costs the same as [1,2,512]), which
    are broadcast to 64 rows via two one-hot PE matmuls (2 waves per
    matmul). No gpsimd in the attention critical path. Out-projections
    spread into the next qc's attention as PE filler; partial outputs
    DMA straight to DRAM.
"""

import sys

if "/opt/trn_rl_repo" not in sys.path:
    sys.path.insert(0, "/opt/trn_rl_repo")

import numpy as np
import ml_dtypes

import concourse.bass as bass
import concourse.tile as tile
from concourse import bacc, mybir
from concourse.bass_utils import run_bass_kernel_spmd
from concourse.masks import make_identity

# Problem constants (hardcoded per harness contract).
B = 4
N = 2048
D_MODEL = 1024
N_HEADS = 16
D_HEAD = 64
ROPE_BASE = 10000.0
EPS = float(np.finfo(np.float32).eps)
N_CORES = 8

HPC = N_HEADS // 2          # heads per core = 8
WAVES = HPC // 2            # head-pair waves = 4
TOKCH = 512                 # token chunk for projections / q chunks
NT = N // TOKCH             # 4
QT = 128                    # query tile for mask classification
NQT = N // QT               # 16
KB = 128                    # key block
NKB = N // KB               # 16
DC = 128                    # dmodel chunk
NDC = D_MODEL // DC         # 8

F32 = mybir.dt.float32
BF16 = mybir.dt.bfloat16
BF = ml_dtypes.bfloat16

ACT = mybir.ActivationFunctionType

# head-dim row permutation: rope partner (d, d+32) -> 16 rows apart within
# one 32-partition quadrant, so the rotate-half swap is a DVE stream_shuffle
PERM64 = np.concatenate(
    [np.arange(0, 16), np.arange(32, 48), np.arange(16, 32), np.arange(48, 64)]
)
SWAP_MASK = list(range(16, 32)) + list(range(0, 16))

_CACHE = {}


def _pin_act_tables(arch):
    """Steer bacc's ACT-table-set choice to natural_log_exp_and_others."""
    from concourse.hw_specs import get_activation_tables

    tables = get_activation_tables(arch)
    keep = "natural_log_exp_and_others"
    if keep not in tables:
        return
    ours = {ACT.Copy, ACT.Square, ACT.Ln, ACT.Exp, ACT.Identity}
    for name, fns in tables.items():
        if name != keep:
            fns -= ours


def _classify_mask(mask):
    """Per (key-block, query-tile) classification of the mask."""
    mask = np.asarray(mask)
    assert mask.shape == (N, N)
    patterns = []
    pat_keys = {}
    state = [[None] * NQT for _ in range(NKB)]
    for kb in range(NKB):
        for qt in range(NQT):
            blk = mask[qt * QT : (qt + 1) * QT, kb * KB : (kb + 1) * KB]
            if blk.all():
                state[kb][qt] = "skip"
            elif not blk.any():
                state[kb][qt] = "full"
            else:
                tileq = (~blk.T).astype(BF)
                key = tileq.tobytes()
                if key not in pat_keys:
                    pat_keys[key] = len(patterns)
                    patterns.append(tileq)
                state[kb][qt] = pat_keys[key]
    return state, patterns


def _build_program(state, n_patterns):
    """Build the SPMD Bass program (same graph on all 8 cores)."""
    nc = bacc.Bacc(
        "TRN2", target_bir_lowering=False, debug=False, num_devices=N_CORES
    )
    _pin_act_tables(nc.m.arch)

    p_xt = nc.dram_tensor("xt", [D_MODEL, N], BF16, kind="ExternalInput").ap()
    p_wqk = nc.dram_tensor("wqk", [128, WAVES, 2, NDC, 128], BF16, kind="ExternalInput").ap()
    p_wv = nc.dram_tensor("wv", [128, WAVES, NDC, 128], BF16, kind="ExternalInput").ap()
    p_wo = nc.dram_tensor("wo", [128, 4, D_MODEL], BF16, kind="ExternalInput").ap()
    p_rope = nc.dram_tensor("rope", [128, 2, N], BF16, kind="ExternalInput").ap()
    p_wcol = nc.dram_tensor("wcol", [128, 2], F32, kind="ExternalInput").ap()
    p_ind2 = nc.dram_tensor("ind2", [128, 2], BF16, kind="ExternalInput").ap()
    p_wfold = nc.dram_tensor("wfold", [2, 128], BF16, kind="ExternalInput").ap()
    p_sel = nc.dram_tensor("sel", [4, 2, 128], BF16, kind="ExternalInput").ap()
    if n_patterns:
        p_pat = nc.dram_tensor(
            "pat", [128, n_patterns, 128], BF16, kind="ExternalInput"
        ).ap()
    p_out = nc.dram_tensor("out", [N, D_MODEL], BF16, kind="ExternalOutput").ap()

    QPC = TOKCH // QT  # query tiles per chunk = 4
    n_kb = [0] * NT
    qlo_t = {}
    for qc in range(NT):
        for kb in range(NKB):
            sub = [state[kb][qc * QPC + j] for j in range(QPC)]
            if all(s == "skip" for s in sub):
                continue
            n_kb[qc] = max(n_kb[qc], kb + 1)
            lead = 0
            while sub[lead] == "skip":
                lead += 1
            qlo_t[(qc, kb)] = lead

    with tile.TileContext(nc) as tc:
        import contextlib

        ctx = contextlib.ExitStack()
        with ctx:
            singles = ctx.enter_context(tc.tile_pool(name="singles", bufs=1))
            wavep = ctx.enter_context(tc.tile_pool(name="wavep", bufs=2))
            invp = ctx.enter_context(tc.tile_pool(name="invp", bufs=2))
            work = ctx.enter_context(tc.tile_pool(name="work", bufs=2))
            espool = ctx.enter_context(tc.tile_pool(name="es", bufs=4))
            epi = ctx.enter_context(tc.tile_pool(name="epi", bufs=2))
            yrp = ctx.enter_context(tc.tile_pool(name="yrp", bufs=2))
            outp = ctx.enter_context(tc.tile_pool(name="outp", bufs=2))

            # PSUM budget (8 banks): tag "s" 3x[128,2,512]f32 = 6 banks,
            # po 1x[128,2,512] = 2 banks.
            ps = ctx.enter_context(tc.tile_pool(name="ps", bufs=3, space="PSUM"))
            ppo = ctx.enter_context(tc.tile_pool(name="ppo", bufs=1, space="PSUM"))

            # ---- resident constants -------------------------------------
            xt_sb = [singles.tile([128, NDC, TOKCH], BF16, name=f"xt{t}") for t in range(NT)]
            wqk_sb = [
                singles.tile([128, 2, NDC, 128], BF16, name=f"wqk{w}")
                for w in range(WAVES)
            ]
            wv_sb = [
                singles.tile([128, NDC, 128], BF16, name=f"wv{w}")
                for w in range(WAVES)
            ]
            rope_sb = singles.tile([128, 2, N], BF16)
            wcol = singles.tile([128, 2], F32)
            ident = singles.tile([128, 128], BF16)
            make_identity(nc, ident)
            eps_sb = singles.tile([128, 1], F32)
            nc.vector.memset(eps_sb, EPS)
            ind2 = singles.tile([128, 2], BF16)
            wfold = singles.tile([2, 128], BF16)
            sel_sb = singles.tile([4, 2, 128], BF16)
            if n_patterns:
                pat_sb = singles.tile([128, n_patterns, 128], BF16)
            yt_sb = [
                singles.tile([128, WAVES, TOKCH], BF16, name=f"yt{qc}")
                for qc in range(NT)
            ]
            wo_sb = singles.tile([128, 4, D_MODEL], BF16)
            qk_rot = [
                singles.tile([128, 2, N], BF16, name=f"qkrot{w}")
                for w in range(WAVES)
            ]
            v_sb = [
                singles.tile([128, NKB, 130], BF16, name=f"vsb{w}")
                for w in range(WAVES)
            ]

            # ---- initial DMAs: large batched transfers, need-order ------
            # sync queue: x chunks (first chunk's tokens lead).
            # gpsimd queue: wave-0 weights first, then smalls, then the rest.
            # scalar queue: late-needed wo (one trigger, doesn't delay ACT).
            for t in range(NT):
                nc.sync.dma_start(
                    out=xt_sb[t],
                    in_=p_xt.rearrange("(dc p) n -> p dc n", p=128)[
                        :, :, t * TOKCH : (t + 1) * TOKCH
                    ],
                )
            for qk in range(2):
                nc.gpsimd.dma_start(out=wqk_sb[0][:, qk], in_=p_wqk[:, 0, qk])
            nc.gpsimd.dma_start(out=wv_sb[0], in_=p_wv[:, 0, :, :])
            nc.gpsimd.dma_start(out=wcol, in_=p_wcol)
            nc.gpsimd.dma_start(out=ind2, in_=p_ind2)
            nc.gpsimd.dma_start(out=wfold, in_=p_wfold)
            nc.gpsimd.dma_start(out=sel_sb, in_=p_sel)
            nc.gpsimd.dma_start(
                out=rope_sb[:, :, 0 : N // 2], in_=p_rope[:, :, 0 : N // 2]
            )
            for w2 in range(1, WAVES):
                nc.gpsimd.dma_start(
                    out=wqk_sb[w2].rearrange("p qk dc f -> p (qk dc f)"),
                    in_=p_wqk[:, w2].rearrange("p qk dc f -> p (qk dc f)"),
                )
                nc.gpsimd.dma_start(out=wv_sb[w2], in_=p_wv[:, w2, :, :])
            nc.gpsimd.dma_start(
                out=rope_sb[:, :, N // 2 :], in_=p_rope[:, :, N // 2 :]
            )
            if n_patterns:
                nc.gpsimd.dma_start(out=pat_sb, in_=p_pat)
            nc.scalar.dma_start(out=wo_sb, in_=p_wo)
            for w in range(WAVES):
                nc.vector.memset(v_sb[w][:, :, 64:65], 1.0)
                nc.vector.memset(v_sb[w][:, :, 129:130], 1.0)

            # =============== P phase: proj + rms + rope ==================
            def emit_P_proj(w, t):
                pj = ps.tile([128, 2, TOKCH], F32, tag="s", name="pj")
                for qk in range(2):
                    for dc in range(NDC):
                        nc.tensor.matmul(
                            pj[:, qk, :],
                            lhsT=wqk_sb[w][:, qk, dc, :],
                            rhs=xt_sb[t][:, dc, :],
                            start=(dc == 0),
                            stop=(dc == NDC - 1),
                        )
                pjv = ps.tile([128, 2, TOKCH], F32, tag="s", name="pjv")
                for dc in range(NDC):
                    nc.tensor.matmul(
                        pjv[:, 0, :],
                        lhsT=wv_sb[w][:, dc, :],
                        rhs=xt_sb[t][:, dc, :],
                        start=(dc == 0),
                        stop=(dc == NDC - 1),
                    )
                return pj, pjv

            def emit_P_evac(w, t, pj, pjv):
                raw = wavep.tile([128, 2, TOKCH], BF16, tag="raw", name="raw")
                for qk in range(2):
                    nc.vector.tensor_scalar_mul(
                        raw[:, qk, :], pj[:, qk, :], wcol[:, qk : qk + 1]
                    )
                sq = work.tile([128, 2, TOKCH], BF16, tag="sq")
                nc.scalar.square(sq, pj)          # ACT
                vt = work.tile([128, TOKCH], BF16, tag="vt")
                nc.scalar.copy(vt, pjv[:, 0, :])  # ACT
                return raw, sq, vt

            def emit_P_rms(w, t, sq):
                lnm = work.tile([2, 2, TOKCH], BF16, tag="qn")
                inv = invp.tile([2, 2, TOKCH], BF16, tag="inv", name="inv")
                ssp = ps.tile([2, 2, TOKCH], F32, tag="s", name="ssp")
                for qk in range(2):
                    nc.tensor.matmul(
                        ssp[:, qk, :], lhsT=ind2, rhs=sq[:, qk, :],
                        start=True, stop=True,
                    )
                nc.scalar.activation(
                    lnm, ssp, ACT.Ln, bias=eps_sb[0:2, :], scale=1.0 / D_HEAD
                )
                nc.scalar.activation(inv, lnm, ACT.Exp, scale=-0.5)
                return inv

            def emit_P_vtrans(w, t, vt):
                ptr = ps.tile([128, 4, 128], BF16, tag="s", name="ptr")
                for sview in range(4):
                    nc.tensor.transpose(
                        ptr[:, sview, :],
                        vt[:, sview * 128 : (sview + 1) * 128],
                        ident,
                    )
                kb0 = t * 4
                nc.vector.tensor_copy(
                    v_sb[w][:, kb0 : kb0 + 4, 0:64], ptr[:, :, 0:64]
                )
                nc.vector.tensor_copy(
                    v_sb[w][:, kb0 : kb0 + 4, 65:129], ptr[:, :, 64:128]
                )

            def emit_P_rope(w, t, raw, inv):
                """fac matmul + rope muls; rotate-half swap on DVE
                stream_shuffle (rows host-permuted)."""
                tsl = slice(t * TOKCH, (t + 1) * TOKCH)
                qn = work.tile([128, 2, TOKCH], BF16, tag="qn")
                qsw = work.tile([128, 2, TOKCH], BF16, tag="qsw")
                fsw = [None, None]
                for qk in range(2):
                    fsw[qk] = ps.tile([128, 2, TOKCH], F32, tag="s", name="fsw")
                    nc.tensor.matmul(
                        fsw[qk][:, 0, :], lhsT=wfold, rhs=inv[:, qk, :],
                        start=True, stop=True,
                    )
                    nc.vector.tensor_mul(
                        qn[:, qk, :], raw[:, qk, :], fsw[qk][:, 0, :]
                    )
                nc.vector.stream_shuffle(qsw, qn, SWAP_MASK)
                for qk in range(2):
                    nc.vector.tensor_mul(
                        qn[:, qk, :], qn[:, qk, :], rope_sb[:, 0, tsl]
                    )
                    nc.vector.tensor_mul(
                        qsw[:, qk, :], qsw[:, qk, :], rope_sb[:, 1, tsl]
                    )
                nc.vector.tensor_add(qk_rot[w][:, :, tsl], qn, qsw)

            # =============== A phase: attention, qc-major ================
            def emit_D(qc, w, prologue):
                kbs = [kb for kb in range(n_kb[qc]) if (qc, kb) in qlo_t]
                po = ppo.tile([128, 2, TOKCH], F32, tag="po", name="po")
                first = [True, True]
                pend = []

                def flush_pv(kb, es, last):
                    qlo = qlo_t[(qc, kb)] * QT
                    osl = slice(qlo, TOKCH)
                    for h2 in range(2):
                        nc.tensor.matmul(
                            po[0:65, h2, osl],
                            lhsT=v_sb[w][:, kb, 65 * h2 : 65 * h2 + 65],
                            rhs=es[:, h2, osl],
                            start=first[h2],
                            stop=last,
                        )
                        first[h2] = False

                for i, kb in enumerate(kbs):
                    qlo = qlo_t[(qc, kb)] * QT
                    csl = slice(qc * TOKCH + qlo, (qc + 1) * TOKCH)
                    osl = slice(qlo, TOKCH)
                    pst = ps.tile([128, 2, TOKCH], F32, tag="s", name="pst")
                    for h2 in range(2):
                        hr = slice(64 * h2, 64 * h2 + 64)
                        nc.tensor.matmul(
                            pst[:, h2, osl],
                            lhsT=qk_rot[w][hr, 1, kb * KB : (kb + 1) * KB],
                            rhs=qk_rot[w][hr, 0, csl],
                            start=True,
                            stop=True,
                        )
                    es = espool.tile([128, 2, TOKCH], BF16, tag="es", name="es")
                    nc.scalar.activation(
                        es[:, :, osl], pst[:, :, osl], ACT.Exp,
                        scale=float(D_HEAD) ** -0.5,
                    )
                    for j in range(qlo // QT, QPC):
                        st = state[kb][qc * QPC + j]
                        if isinstance(st, int):
                            jsl = slice(j * QT, (j + 1) * QT)
                            for h2 in range(2):
                                nc.vector.tensor_mul(
                                    es[:, h2, jsl], es[:, h2, jsl],
                                    pat_sb[:, st, :],
                                )
                    if prologue:
                        prologue.pop(0)()
                    pend.append((kb, es))
                    if len(pend) > 2:
                        k0, e0 = pend.pop(0)
                        flush_pv(k0, e0, False)
                for fn in prologue:
                    fn()
                for i, (k0, e0) in enumerate(pend):
                    flush_pv(k0, e0, i == len(pend) - 1)

                # per-wave epilogue half: evacuate y rows + stage the
                # denominator row onto partition w of the qc's den tile
                yr = yrp.tile([64, 2, TOKCH], BF16, tag=f"yr{w}", name="yr")
                nc.vector.tensor_copy(yr, po[0:64, :, :])
                denw = epi.tile([1, 2, TOKCH], BF16, tag="denw", name="denw")
                nc.vector.tensor_copy(denw, po[64:65, :, :])
                return yr, denw

            def make_epi_qc(qc, yrs, dens, pden):
                """qc-level epilogue: one Ln+Exp for all 4 waves, PE one-hot
                broadcast of the reciprocals, DVE yt multiplies."""
                def fn():
                    lnp = epi.tile([4, 2, TOKCH], F32, tag="lnp", name="lnp")
                    nc.scalar.activation(lnp, pden, ACT.Ln)
                    recb = epi.tile([4, 2, TOKCH], BF16, tag="recb", name="recb")
                    nc.scalar.activation(recb, lnp, ACT.Exp, scale=-1.0)
                    for pair in range(2):
                        f2 = ps.tile([128, 2, TOKCH], F32, tag="s", name="f2")
                        for h2 in range(2):
                            nc.tensor.matmul(
                                f2[:, h2, :], lhsT=sel_sb[:, pair, :],
                                rhs=recb[:, h2, :],
                                start=True, stop=True,
                            )
                        for wi in range(2):
                            w = pair * 2 + wi
                            for h2 in range(2):
                                nc.vector.tensor_mul(
                                    yt_sb[qc][64 * h2 : 64 * h2 + 64, w, :],
                                    yrs[w][:, h2, :],
                                    f2[64 * wi : 64 * wi + 64, h2, :],
                                )
                return fn

            def out_unit(qc, i):
                def fn():
                    pot = ps.tile([128, 2, TOKCH], F32, tag="s", name="pot")
                    for ec in range(2):
                        for fc in range(4):
                            nc.tensor.matmul(
                                pot[:, ec, :],
                                lhsT=yt_sb[qc][:, fc, i * 128 : (i + 1) * 128],
                                rhs=wo_sb[:, fc, ec * TOKCH : (ec + 1) * TOKCH],
                                start=(fc == 0),
                                stop=(fc == 3),
                            )
                    osb = outp.tile([128, 2, TOKCH], BF16, tag="o", name="osb")
                    nc.vector.tensor_copy(osb, pot)
                    nc.sync.dma_start(
                        out=p_out[qc * TOKCH + i * 128 : qc * TOKCH + (i + 1) * 128, :],
                        in_=osb,
                    )
                return fn

            # ---------------- emission schedule --------------------------
            chunks = [(w, t) for w in range(WAVES) for t in range(NT)]
            pend = None
            for w, t in chunks:
                if pend is not None:
                    pw, pt, raw_p, sq_p, vt_p = pend
                    inv_p = emit_P_rms(pw, pt, sq_p)
                pj, pjv = emit_P_proj(w, t)
                raw, sq, vt = emit_P_evac(w, t, pj, pjv)
                if pend is not None:
                    emit_P_vtrans(pw, pt, vt_p)
                    emit_P_rope(pw, pt, raw_p, inv_p)
                pend = (w, t, raw, sq, vt)
            pw, pt, raw_p, sq_p, vt_p = pend
            inv_p = emit_P_rms(pw, pt, sq_p)
            emit_P_vtrans(pw, pt, vt_p)
            emit_P_rope(pw, pt, raw_p, inv_p)

            # A phase: per-qc den staging tile; epilogue+out-proj of qc
            # spread into qc+1's attention as PE filler
            filler = []
            for qc in [0, 1, 2, 3]:
                yrs, dens = [], []
                pden = epi.tile([4, 2, TOKCH], BF16, tag="pden", name="pden")
                for w in range(WAVES):
                    prologue = []
                    for _ in range(2):
                        if filler:
                            prologue.append(filler.pop(0))
                    yr, denw = emit_D(qc, w, prologue)
                    yrs.append(yr)
                    dens.append(denw)
                    nc.sync.dma_start(out=pden[w : w + 1, :, :], in_=denw)
                assert not filler, f"filler left over at qc={qc}"
                filler = [make_epi_qc(qc, yrs, dens, pden)] + [
                    out_unit(qc, i) for i in range(4)
                ]
                if qc == NT - 1:
                    for fn in filler:
                        fn()
                    filler = []

    nc.compile()
    return nc


def _host_prep(x, mask, pos, W_qkv, W_out, qn_w, kn_w):
    x = np.asarray(x, dtype=np.float32)
    mask = np.asarray(mask)
    pos = np.asarray(pos).astype(np.float64)
    W_qkv = np.asarray(W_qkv, dtype=np.float32)
    W_out = np.asarray(W_out, dtype=np.float32)
    qn_w = np.asarray(qn_w, dtype=np.float32)
    kn_w = np.asarray(kn_w, dtype=np.float32)

    inv_freq = 1.0 / (ROPE_BASE ** (np.arange(0, D_HEAD, 2, dtype=np.float64) / D_HEAD))
    ang = pos[:, None] * inv_freq[None, :]  # (N, 32)
    cosT = np.cos(ang).T.astype(np.float32)  # (32, N)
    sinT = np.sin(ang).T.astype(np.float32)

    # permuted-row rope tables: 64-block layout is
    # [t1 dims 0:16, t2 dims 0:16, t1 dims 16:32, t2 dims 16:32]
    cos64 = np.concatenate([cosT[0:16], cosT[0:16], cosT[16:32], cosT[16:32]], axis=0)
    sin64 = np.concatenate([-sinT[0:16], sinT[0:16], -sinT[16:32], sinT[16:32]], axis=0)
    cos_d = np.tile(cos64, (2, 1))
    sin_d = np.tile(sin64, (2, 1))
    rope = np.stack([cos_d, sin_d], axis=1).astype(BF)  # (128, 2, N)

    qn_p = qn_w[PERM64]
    kn_p = kn_w[PERM64]
    wcol_np = np.stack([np.tile(qn_p, 2), np.tile(kn_p, 2)], axis=1).astype(
        np.float32
    )  # (128, 2)

    ind2_np = np.zeros((128, 2), dtype=np.float32)
    ind2_np[0:64, 0] = 1.0
    ind2_np[64:128, 1] = 1.0
    ind2_np = ind2_np.astype(BF)
    wfold_np = np.ascontiguousarray(ind2_np.T)  # (2, 128)

    # one-hot wave-pair selectors for the reciprocal broadcast:
    # sel[:, pair, :]: [4, 128] with rows (2*pair+wi) -> cols 64*wi..64*wi+64
    sel_np = np.zeros((4, 2, 128), dtype=np.float32)
    for pair in range(2):
        for wi in range(2):
            sel_np[pair * 2 + wi, pair, 64 * wi : 64 * wi + 64] = 1.0
    sel_np = sel_np.astype(BF)

    state, patterns = _classify_mask(mask)
    if patterns:
        pat = np.stack(patterns, axis=1).astype(BF)
    else:
        pat = None

    q_rows = lambda h: slice(h * 192, h * 192 + 64)
    k_rows = lambda h: slice(h * 192 + 64, h * 192 + 128)
    v_rows = lambda h: slice(h * 192 + 128, h * 192 + 192)

    in_maps = []
    for c in range(N_CORES):
        b, half = divmod(c, 2)
        hs = [8 * half + i for i in range(8)]
        # permuted q/k head-dim rows
        wqk = np.concatenate(
            [W_qkv[q_rows(h)][PERM64] for h in hs]
            + [W_qkv[k_rows(h)][PERM64] for h in hs],
            axis=0,
        ).T  # (1024 dmodel, 1024 cols)
        wv = np.concatenate([W_qkv[v_rows(h)] for h in hs], axis=0).T
        wo = W_out[:, 512 * half : 512 * half + 512].T  # (512, 1024)
        # (128, WAVES, 2, NDC, 128): [p, w, qk, dc, f]
        wqk_re = np.ascontiguousarray(
            wqk.reshape(NDC, 128, 2, WAVES, 128).transpose(1, 3, 2, 0, 4)
        )
        wv_re = np.ascontiguousarray(
            wv.reshape(NDC, 128, WAVES, 128).transpose(1, 2, 0, 3)
        )
        wo_re = np.ascontiguousarray(wo.reshape(4, 128, 1024).transpose(1, 0, 2))
        m = {
            "xt": np.ascontiguousarray(x[b].T).astype(BF),
            "wqk": wqk_re.astype(BF),
            "wv": wv_re.astype(BF),
            "wo": wo_re.astype(BF),
            "rope": rope,
            "wcol": wcol_np,
            "ind2": ind2_np,
            "wfold": wfold_np,
            "sel": sel_np,
        }
        if pat is not None:
            m["pat"] = pat
        in_maps.append(m)
    return in_maps, state, (0 if pat is None else pat.shape[1])


def kernel(x, mask, pos, W_qkv, W_out, qn_w, kn_w, _trace=False):
    in_maps, state, n_pat = _host_prep(x, mask, pos, W_qkv, W_out, qn_w, kn_w)
    key = (str(state), n_pat)
    if key not in _CACHE:
        _CACHE[key] = _build_program(state, n_pat)
    nc = _CACHE[key]
    res = run_bass_kernel_spmd(nc, in_maps, list(range(N_CORES)), trace=_trace)
    out = np.empty((B, N, D_MODEL), dtype=np.float32)
    for b in range(B):
        lo = res.results[2 * b]["out"].astype(np.float32)
        hi = res.results[2 * b + 1]["out"].astype(np.float32)
        out[b] = lo + hi
    kernel._last_results = res
    return out
